# revision 1
# baseline (speedup 1.0000x reference)
"""Trainium2 Bass kernel for nn_Encoder_79585743995180 (sparse_attention).

Self-contained: hardcodes shapes/sharding. Strategy (validated in numpy):
  - 8 cores, head-parallel: core c owns heads {2c, 2c+1} (128 of 1024 dims).
  - Per core: q/k/v projections for its 128 dims (reads full activations,
    sliced weights), rope (de-interleaved even/odd permutation so the
    rotation partner sits at partition offset +32 within each 64-dim head
    block), main attention with column-softmax folded into a 1/colsum
    prescale of the AV stationary operand, memory attention with mask+gate
    folded into the host-prepped vmaug tensor, out_proj partial product.
  - Host sums the 8 partial outputs (contraction-sharded out_proj).
  - Matmul operands in fp16 (fp32 matmuls are split into hi/lo passes on
    trn2 PE = 2x instructions; fp16 has 4x the mantissa precision of bf16
    at the same PE rate); accumulation stays fp32 in PSUM, and the softmax
    renormalization path stays fp32.

All biases in this problem are zeros (spec fill=zeros) and are skipped.
The reference's `+1e-8` softmax epsilon is omitted (validated: rel err
~4e-6 vs reference in fp32).

Layout conventions on device (per core):
  qT/kT   (128 dims, 4096 rows) bf16   rows r = n*L + l, dims rope-permuted
  v       rows layout, stored as v_sb (128 rows%128, 32 rowtile, 2 head, 65)
          bf16, with ones in column 64 (renorm denominator rides the AV mm)
  attnT   (128 dims, 4096 rows) bf16
  outT    (1024, 4096) fp32 partial, host sums across cores.
"""

import ml_dtypes
import numpy as np

import concourse.bacc as bacc
import concourse.mybir as mybir
import concourse.tile as tile
from concourse import bass_utils

F32 = mybir.dt.float32
BF16 = mybir.dt.float16
NPBF = np.float16
AF = mybir.ActivationFunctionType

L = 1024
S = 1024
N = 4
E = 1024
H = 16
D = 64
M = 512
NC = 8
HPC = H // NC          # 2 heads per core
DC = HPC * D           # 128 dims per core
R = L * N              # 4096 rows, r = n*L + l

_COMPILED = {}


def _build(dbg=False):
    nc = bacc.Bacc("TRN2", target_bir_lowering=False, debug=False)

    # ---- DRAM I/O ----
    xqT = nc.dram_tensor("xqT", [E, R], BF16, kind="ExternalInput").ap()
    xkT = nc.dram_tensor("xkT", [E, R], BF16, kind="ExternalInput").ap()
    xvT = nc.dram_tensor("xvT", [E, R], BF16, kind="ExternalInput").ap()
    wqT = nc.dram_tensor("wqT", [E, DC], BF16, kind="ExternalInput").ap()
    wkT = nc.dram_tensor("wkT", [E, DC], BF16, kind="ExternalInput").ap()
    wvT = nc.dram_tensor("wvT", [E, DC], BF16, kind="ExternalInput").ap()
    woT = nc.dram_tensor("woT", [DC, E], BF16, kind="ExternalInput").ap()
    cosq = nc.dram_tensor("cosq", [DC, R], BF16, kind="ExternalInput").ap()
    sinq = nc.dram_tensor("sinq", [DC, R], BF16, kind="ExternalInput").ap()
    cosk = nc.dram_tensor("cosk", [DC, R], BF16, kind="ExternalInput").ap()
    sink = nc.dram_tensor("sink", [DC, R], BF16, kind="ExternalInput").ap()
    kmem = nc.dram_tensor("kmem", [DC, N, M], BF16, kind="ExternalInput").ap()
    vmaug = nc.dram_tensor("vmaug", [128, N, HPC, 4, 65], BF16,
                           kind="ExternalInput").ap()
    outT = nc.dram_tensor("outT", [E, R], BF16, kind="ExternalOutput").ap()
    dbg_t = {}
    if dbg:
        for nm, shp in (("dbg_q", [DC, R]), ("dbg_k", [DC, R]),
                        ("dbg_attn", [DC, R])):
            dbg_t[nm] = nc.dram_tensor(nm, shp, F32, kind="ExternalOutput").ap()

    with tile.TileContext(nc) as tc:
        with (
            tc.tile_pool(name="const", bufs=1) as const,
            tc.tile_pool(name="persist", bufs=1) as persist,
            tc.tile_pool(name="xstream", bufs=3) as xstream,
            tc.tile_pool(name="cs", bufs=2) as cs,
            tc.tile_pool(name="scratch", bufs=2) as scratch,
            tc.tile_pool(name="attnscr", bufs=2) as attnscr,
            tc.tile_pool(name="rows", bufs=1) as rows,
            tc.tile_pool(name="drows", bufs=4, space="DRAM") as drows,
            tc.tile_pool(name="wexp", bufs=11) as wexpp,
            tc.tile_pool(name="small", bufs=3) as small,
            tc.tile_pool(name="ostage", bufs=3) as ostage,
            tc.tile_pool(name="pw", bufs=2, space="PSUM") as pw,
            tc.tile_pool(name="pproj", bufs=2, space="PSUM") as pproj,
            tc.tile_pool(name="pacc", bufs=1, space="PSUM") as pacc,
        ):
            # ---- constants into SBUF ----
            w_sb = {}
            for name, src in (("q", wqT), ("k", wkT), ("v", wvT)):
                t = const.tile([128, 8, DC], BF16, tag=f"w_{name}")
                nc.sync.dma_start(
                    out=t, in_=src.rearrange("(kc p) d -> p kc d", p=128))
                w_sb[name] = t
            wo_sb = const.tile([DC, E], BF16)
            nc.sync.dma_start(out=wo_sb, in_=woT)
            kmem_sb = const.tile([DC, N, M], BF16)
            nc.sync.dma_start(out=kmem_sb, in_=kmem)
            vmaug_sb = const.tile([128, N, HPC, 4, 65], BF16)
            nc.sync.dma_start(out=vmaug_sb, in_=vmaug)

            # per-n persistent tiles so Tile can pipeline proj(n+1)
            # under attn/outproj(n)
            qT_n = [persist.tile([DC, L], BF16, tag=f"qT{n}", name=f"qT{n}") for n in range(N)]
            kT_n = [persist.tile([DC, L], BF16, tag=f"kT{n}", name=f"kT{n}") for n in range(N)]
            v_n = [persist.tile([128, 8, HPC, 65], BF16, tag=f"v{n}",
                                name=f"v{n}") for n in range(N)]
            attn_n = [persist.tile([DC, L], BF16, tag=f"at{n}",
                                   name=f"at{n}") for n in range(N)]
            for n in range(N):
                nc.vector.memset(v_n[n][:, :, :, 64:65], 1.0)

            def emit_proj(n):
                # ---- projections for batch n (rows n*L .. n*L+L) ----
                nrows = slice(n * L, (n + 1) * L)
                for name, xT, cosT, sinT in (
                    ("q", xqT, cosq, sinq),
                    ("k", xkT, cosk, sink),
                ):
                    dest = qT_n[n] if name == "q" else kT_n[n]
                    xs = xstream.tile([128, 8, 1024], BF16, tag="xs")
                    nc.sync.dma_start(
                        out=xs,
                        in_=xT[:, nrows].rearrange("(kc p) r -> p kc r", p=128))
                    ctw = cs.tile([128, 1024], BF16, tag="ct")
                    stw = cs.tile([128, 1024], BF16, tag="st")
                    nc.sync.dma_start(out=ctw, in_=cosT[:, nrows])
                    nc.sync.dma_start(out=stw, in_=sinT[:, nrows])
                    for rt2 in range(2):
                        ls = slice(rt2 * 512, (rt2 + 1) * 512)
                        ps = pproj.tile([128, 512], F32, tag="pp")
                        for kc in range(8):
                            nc.tensor.matmul(
                                ps, w_sb[name][:, kc, :], xs[:, kc, ls],
                                start=(kc == 0), stop=(kc == 7))
                        t1 = scratch.tile([128, 512], BF16, tag="t1")
                        nc.vector.tensor_mul(t1, ps, ctw[:, ls])
                        z = scratch.tile([128, 512], BF16, tag="z")
                        nc.vector.tensor_mul(z, ps, stw[:, ls])
                        t2 = scratch.tile([128, 512], BF16, tag="t2")
                        for hb in range(HPC):
                            b = hb * 64
                            nc.gpsimd.dma_start(
                                out=t2[b:b + 32, :], in_=z[b + 32:b + 64, :])
                            nc.gpsimd.dma_start(
                                out=t2[b + 32:b + 64, :], in_=z[b:b + 32, :])
                        nc.vector.tensor_add(dest[:, ls], t1, t2)
                # v projection for batch n
                xs = xstream.tile([128, 8, 1024], BF16, tag="xs")
                nc.sync.dma_start(
                    out=xs,
                    in_=xvT[:, nrows].rearrange("(kc p) r -> p kc r", p=128))
                for st_i in range(8):
                    ps = pproj.tile([128, 512], F32, tag="pp")
                    for kc in range(8):
                        nc.tensor.matmul(
                            ps[:, 0:128],
                            xs[:, kc, st_i * 128:(st_i + 1) * 128],
                            w_sb["v"][:, kc, :],
                            start=(kc == 0), stop=(kc == 7))
                    for h in range(HPC):
                        nc.scalar.activation(
                            v_n[n][:, st_i, h, 0:64],
                            ps[:, h * 64:(h + 1) * 64], AF.Copy)


            def emit_attn_out(n):
                # ---- attention for batch n, both heads ----
                for h in range(HPC):
                    ho = h * 64
                    colsum = small.tile([128, 8], F32, tag="colsum")
                    wxs = []
                    for sc in range(8):
                        pwt = pw.tile([128, 1024], F32, tag="pw")
                        for lc in range(2):
                            nc.tensor.matmul(
                                pwt[:, lc * 512:(lc + 1) * 512],
                                kT_n[n][ho:ho + 64,
                                        sc * 128:(sc + 1) * 128],
                                qT_n[n][ho:ho + 64,
                                        lc * 512:(lc + 1) * 512],
                                start=True, stop=True)
                        wx = wexpp.tile([128, 1024], BF16, tag="wx")
                        nc.scalar.activation(
                            wx, pwt, AF.Exp, accum_out=colsum[:, sc:sc + 1])
                        wxs.append(wx)
                    rcall = small.tile([128, 8], F32, tag="rcall")
                    nc.vector.reciprocal(rcall, colsum)
                    pmain = pacc.tile([65, 1024], F32, tag="pmain")
                    for sc in range(8):
                        vs = small.tile([128, 65], BF16, tag="vs")
                        nc.vector.tensor_scalar_mul(
                            vs, v_n[n][:, sc, h, :], rcall[:, sc:sc + 1])
                        for lc in range(2):
                            nc.tensor.matmul(
                                pmain[:, lc * 512:(lc + 1) * 512],
                                vs, wxs[sc][:, lc * 512:(lc + 1) * 512],
                                start=(sc == 0), stop=(sc == 7))
                    pmem = pacc.tile([65, 1024], F32, tag="pmain")
                    for mc in range(4):
                        pwt = pw.tile([128, 1024], F32, tag="pw")
                        for lc in range(2):
                            nc.tensor.matmul(
                                pwt[:, lc * 512:(lc + 1) * 512],
                                kmem_sb[ho:ho + 64, n,
                                        mc * 128:(mc + 1) * 128],
                                qT_n[n][ho:ho + 64,
                                        lc * 512:(lc + 1) * 512],
                                start=True, stop=True)
                        wx = wexpp.tile([128, 1024], BF16, tag="wx")
                        nc.scalar.activation(wx, pwt, AF.Exp)
                        for lc in range(2):
                            nc.tensor.matmul(
                                pmem[:, lc * 512:(lc + 1) * 512],
                                vmaug_sb[:, n, h, mc, :],
                                wx[:, lc * 512:(lc + 1) * 512],
                                start=(mc == 0), stop=(mc == 3))
                    smain = attnscr.tile([65, 1024], F32, tag="smain")
                    smem = attnscr.tile([65, 1024], F32, tag="smem")
                    nc.scalar.activation(smain, pmain, AF.Copy)
                    nc.vector.tensor_copy(smem, pmem)
                    d1 = rows.tile([1, 1024], F32, tag="d1")
                    d2 = rows.tile([1, 1024], F32, tag="d2")
                    nc.gpsimd.dma_start(out=d1, in_=smain[64:65, :])
                    nc.gpsimd.dma_start(out=d2, in_=smem[64:65, :])
                    # out = (smain*D2 + smem*D1) / (D1*D2): one row recip
                    m12 = rows.tile([1, 1024], F32, tag="m12")
                    nc.vector.tensor_mul(m12, d1, d2)
                    w12 = rows.tile([1, 1024], F32, tag="w12")
                    nc.vector.reciprocal(w12, m12)
                    r1 = rows.tile([1, 1024], F32, tag="r1")
                    r2 = rows.tile([1, 1024], F32, tag="r2")
                    nc.vector.tensor_mul(r1, d2, w12)   # = 1/D1
                    nc.vector.tensor_mul(r2, d1, w12)   # = 1/D2
                    dr1 = drows.tile([1, 1024], F32, tag="dr1")
                    dr2 = drows.tile([1, 1024], F32, tag="dr2")
                    nc.gpsimd.dma_start(out=dr1, in_=r1)
                    nc.gpsimd.dma_start(out=dr2, in_=r2)
                    bc1 = attnscr.tile([64, 1024], F32, tag="bc1")
                    bc2 = attnscr.tile([64, 1024], F32, tag="bc2")
                    nc.gpsimd.dma_start(
                        out=bc1, in_=dr1.to_broadcast((64, 1024)))
                    nc.gpsimd.dma_start(
                        out=bc2, in_=dr2.to_broadcast((64, 1024)))
                    u1 = attnscr.tile([64, 1024], BF16, tag="u1")
                    nc.vector.tensor_mul(u1, smain[0:64, :], bc1)
                    u2 = attnscr.tile([64, 1024], BF16, tag="u2")
                    nc.vector.tensor_mul(u2, smem[0:64, :], bc2)
                    nc.vector.tensor_add(attn_n[n][ho:ho + 64, :], u1, u2)

                if dbg:
                    nc.sync.dma_start(
                        out=dbg_t["dbg_q"][:, n * L:(n + 1) * L], in_=qT_n[n])
                    nc.sync.dma_start(
                        out=dbg_t["dbg_k"][:, n * L:(n + 1) * L], in_=kT_n[n])
                    nc.sync.dma_start(
                        out=dbg_t["dbg_attn"][:, n * L:(n + 1) * L],
                        in_=attn_n[n])

                # ---- out_proj partial for batch n ----
                for oc in range(8):
                    for rt2 in range(2):
                        po = pproj.tile([128, 512], F32, tag="pp")
                        nc.tensor.matmul(
                            po, wo_sb[:, oc * 128:(oc + 1) * 128],
                            attn_n[n][:, rt2 * 512:(rt2 + 1) * 512],
                            start=True, stop=True)
                        so = ostage.tile([128, 512], BF16, tag="so")
                        dst = outT[oc * 128:(oc + 1) * 128,
                                   n * L + rt2 * 512:n * L + (rt2 + 1) * 512]
                        if (oc * 2 + rt2) % 2 == 0:
                            nc.scalar.activation(so, po, AF.Copy)
                            nc.scalar.dma_start(out=dst, in_=so)
                        else:
                            nc.vector.tensor_copy(so, po)
                            nc.gpsimd.dma_start(out=dst, in_=so)


            emit_proj(0)
            for n in range(N):
                if n + 1 < N:
                    emit_proj(n + 1)
                emit_attn_out(n)
    nc.compile()
    return nc


def _perm64():
    p = np.empty(64, np.int64)
    p[:32] = np.arange(0, 64, 2)
    p[32:] = np.arange(1, 64, 2)
    return p


def _prep_inputs(inputs):
    """Host-side shard prep. Returns list of per-core input dicts."""
    f = np.float32
    query = np.asarray(inputs["query"], f)
    key = np.asarray(inputs["key"], f)
    value = np.asarray(inputs["value"], f)
    W = np.asarray(inputs["in_proj_weight"], f)
    wo = np.asarray(inputs["out_proj_weight"], f)
    qp = np.asarray(inputs["qp"], f)
    kvp = np.asarray(inputs["kvp"], f)
    k_mem = np.asarray(inputs["k_mem"], f)
    v_mem = np.asarray(inputs["v_mem"], f)
    gate = np.asarray(inputs["gate_attn"], f)
    mask = np.asarray(inputs["mem_mask"]).astype(f)

    g = 1.0 / (1.0 + np.exp(-gate))
    perm64 = _perm64()
    sgn = np.concatenate([np.full(32, -1.0, f), np.full(32, 1.0, f)] * HPC)

    xqT = np.ascontiguousarray(
        query.transpose(2, 1, 0).reshape(E, R)).astype(NPBF)
    xkT = np.ascontiguousarray(
        key.transpose(2, 1, 0).reshape(E, R)).astype(NPBF)
    xvT = np.ascontiguousarray(
        value.transpose(2, 1, 0).reshape(E, R)).astype(NPBF)

    in_maps = []
    for c in range(NC):
        dims = np.arange(c * DC, (c + 1) * DC)
        dims_perm = np.concatenate([dims[h * 64 + perm64] for h in range(HPC)])
        gv = np.concatenate(
            [np.full(64, 1.0 - g[2 * c + h], f) for h in range(HPC)])

        wq = W[:E][dims_perm] * np.float32(D ** -0.5)
        wk = W[E:2 * E][dims_perm]
        wv = W[2 * E:][dims] * gv[:, None]

        def rope(pe):
            cosT = np.ascontiguousarray(
                pe[:, :, dims_perm, 0].transpose(2, 0, 1).reshape(DC, R))
            sinT = (pe[:, :, dims_perm, 1].transpose(2, 0, 1).reshape(DC, R)
                    * sgn[:, None])
            # device computes z = qraw * sin then swaps partner rows, so the
            # sin tensor itself must be pre-swapped: st[p] = sin_signed[partner(p)]
            sw = np.empty_like(sinT)
            for hb in range(HPC):
                b = hb * 64
                sw[b:b + 32] = sinT[b + 32:b + 64]
                sw[b + 32:b + 64] = sinT[b:b + 32]
            return cosT.astype(NPBF), np.ascontiguousarray(sw).astype(NPBF)

        cq, sq = rope(qp)
        ck, sk = rope(kvp)

        kmemT = np.ascontiguousarray(
            k_mem[:, dims_perm, :].transpose(1, 0, 2)).astype(NPBF)

        vma = np.zeros((N, HPC, M, 65), f)
        for n in range(N):
            for h in range(HPC):
                gh = g[2 * c + h]
                vm = v_mem[n, dims[h * 64:(h + 1) * 64], :].T  # (M, 64)
                vma[n, h, :, :64] = vm * gh * mask[n][:, None]
                vma[n, h, :, 64] = mask[n]
        vma_dev = np.ascontiguousarray(
            vma.reshape(N, HPC, 4, 128, 65).transpose(3, 0, 1, 2, 4)).astype(NPBF)

        in_maps.append({
            "xqT": xqT, "xkT": xkT, "xvT": xvT,
            "wqT": np.ascontiguousarray(wq.T).astype(NPBF),
            "wkT": np.ascontiguousarray(wk.T).astype(NPBF),
            "wvT": np.ascontiguousarray(wv.T).astype(NPBF),
            "woT": np.ascontiguousarray(wo[:, dims].T).astype(NPBF),
            "cosq": cq, "sinq": sq, "cosk": ck, "sink": sk,
            "kmem": kmemT, "vmaug": vma_dev,
        })
    return in_maps


def kernel(**inputs):
    if "nc" not in _COMPILED:
        _COMPILED["nc"] = _build()
    nc = _COMPILED["nc"]
    in_maps = _prep_inputs(inputs)
    res = bass_utils.run_bass_kernel_spmd(nc, in_maps, core_ids=list(range(NC)))
    total = np.zeros((E, R), np.float64)
    for r in res.results:
        total += r["outT"].astype(np.float64)
    out = total.T.reshape(N, L, E).transpose(1, 0, 2).astype(np.float32)
    out = out + np.asarray(inputs["out_proj_bias"], np.float32)
    return out



# revision 21
# speedup vs baseline: 1.0584x; 1.0584x over previous
"""Trainium2 Bass kernel for nn_Encoder_79585743995180 (sparse_attention).

v2 — batch x head-group sharding. Core c -> (batch n = c//2, head-group
g = c%2 owning 8 heads / 512 dims). vs v1 (head-only sharding):
  - per-core x DMA drops 24MB -> 6MB (each core reads only its batch);
  - projections/attention/outproj all for 8 heads of one batch;
  - renorm path rebuilt: reciprocal_approx_fast (5x faster than
    InstReciprocal), Pool partition_broadcast instead of DRAM-roundtrip
    broadcast DMAs, no [1,1024] multiplies;
  - matmul stream ordered so the PE never waits on the softmax chain
    (QK/memQK of head h+1 emitted before AV of head h).

All matmul operands fp16 (fp8 validated too lossy: >2e-2). fp32 PSUM.
Math per head (validated in numpy, rel err ~7.7e-4):
  z[s,l] = k_h^T q_h   (q pre-scaled by D^-0.5, rope'd)
  wx = exp(z)
  colsum[s] = sum_l wx ; rcall = 1/colsum
  vs[s,:] = [v_h[s,:] , 1] * rcall   -> AV gives numer[d,l], den[l]
  mem path: wxm = exp(zm), vm cols carry gate*mask, col 64 = mask
  attn_h = numer/den + numer_m/den_m   (division via bc'd reciprocal rows)
out_core[e,l] = sum_dc wo[dc,e] attn[dc,l]; host sums the 2 cores per batch.
"""

import numpy as np

import concourse.bacc as bacc
import concourse.mybir as mybir
import concourse.tile as tile
from concourse import bass_utils

F32 = mybir.dt.float32
F16 = mybir.dt.float16
NPF16 = np.float16
AF = mybir.ActivationFunctionType

L = 1024
S = 1024
N = 4
E = 1024
H = 16
D = 64
M = 512
NC = 8
HPC = 8              # heads per core
DCC = HPC * D        # 512 dims per core

_COMPILED = {}


def _build(dbg=False):
    nc = bacc.Bacc("TRN2", target_bir_lowering=False, debug=False)

    # ---- DRAM I/O (all host-prechunked to [128, ...] partition layouts) ----
    xq = nc.dram_tensor("xq", [128, 8, L], F16, kind="ExternalInput").ap()
    xk = nc.dram_tensor("xk", [128, 8, L], F16, kind="ExternalInput").ap()
    xv = nc.dram_tensor("xv", [128, 8, L], F16, kind="ExternalInput").ap()
    wq = nc.dram_tensor("wq", [128, 8, DCC], F16, kind="ExternalInput").ap()
    wk = nc.dram_tensor("wk", [128, 8, DCC], F16, kind="ExternalInput").ap()
    wv = nc.dram_tensor("wv", [128, 8, DCC], F16, kind="ExternalInput").ap()
    wo = nc.dram_tensor("wo", [128, 4, E], F16, kind="ExternalInput").ap()
    cosq = nc.dram_tensor("cosq", [128, 4, L], F16, kind="ExternalInput").ap()
    sinq = nc.dram_tensor("sinq", [128, 4, L], F16, kind="ExternalInput").ap()
    cosk = nc.dram_tensor("cosk", [128, 4, L], F16, kind="ExternalInput").ap()
    sink = nc.dram_tensor("sink", [128, 4, L], F16, kind="ExternalInput").ap()
    kmem = nc.dram_tensor("kmem", [128, 4, M], F16, kind="ExternalInput").ap()
    vm = nc.dram_tensor("vm", [128, HPC, 4, 65], F16, kind="ExternalInput").ap()
    outT = nc.dram_tensor("outT", [128, 8, L], F16, kind="ExternalOutput").ap()
    dbg_t = {}
    if dbg:
        for nm, shp, dt in (("dbg_q", [128, 4, L], F16),
                            ("dbg_k", [128, 4, L], F16),
                            ("dbg_v", [128, 8, HPC, 65], F16),
                            ("dbg_attn", [128, 4, L], F16),
                            ("dbg_colsum", [128, HPC, 8], F32),
                            ("dbg_rcall", [128, HPC, 8], F32),
                            ("dbg_r1", [1, HPC, L], F32),
                            ("dbg_r2", [1, HPC, L], F32),
                            ("dbg_pmain", [65, L], F32),
                            ("dbg_pmem", [65, L], F32),
                            ("dbg_wx", [128, 512], F16)):
            dbg_t[nm] = nc.dram_tensor(nm, shp, dt, kind="ExternalOutput").ap()

    with tile.TileContext(nc) as tc:
        with (
            tc.tile_pool(name="const", bufs=1) as const,
            tc.tile_pool(name="persist", bufs=1) as persist,
            tc.tile_pool(name="wexp", bufs=17 if dbg else 20) as wexpp,
            tc.tile_pool(name="cs", bufs=1 if dbg else 2) as csp,
            tc.tile_pool(name="small", bufs=3) as small,
            tc.tile_pool(name="rows", bufs=1) as rows,
            tc.tile_pool(name="bcp", bufs=1) as bcp,
            tc.tile_pool(name="uscr", bufs=2) as uscr,
            tc.tile_pool(name="rscr", bufs=3 if dbg else 4) as rscr,
            tc.tile_pool(name="ostage", bufs=2 if dbg else 4) as ostage,
            tc.tile_pool(name="pq", bufs=4, space="PSUM") as pq,
            tc.tile_pool(name="pmain", bufs=1, space="PSUM") as pmainp,
            tc.tile_pool(name="pmem", bufs=1, space="PSUM") as pmemp,
        ):
            # ---- constants / activations into SBUF ----
            w_sb = {}
            for name, src in (("q", wq), ("k", wk), ("v", wv)):
                t = const.tile([128, 8, DCC], F16, tag=f"w_{name}")
                nc.sync.dma_start(out=t, in_=src)
                w_sb[name] = t
            wo_sb = const.tile([128, 4, E], F16, tag="wo")
            nc.sync.dma_start(out=wo_sb, in_=wo)
            kmem_sb = const.tile([128, 4, M], F16, tag="kmem")
            nc.sync.dma_start(out=kmem_sb, in_=kmem)
            vm_sb = const.tile([128, HPC, 4, 65], F16, tag="vm")
            nc.sync.dma_start(out=vm_sb, in_=vm)
            cs_src = {"cq": cosq, "sq": sinq, "ck": cosk, "sk": sink}
            # x tensors resident, per-kc chunk DMAs so proj can start early
            x_sb = {}
            for name, src in (("q", xq), ("k", xk), ("v", xv)):
                t = const.tile([128, 8, L], F16, tag=f"x_{name}")
                for kc in range(8):
                    nc.sync.dma_start(out=t[:, kc, :], in_=src[:, kc, :])
                x_sb[name] = t

            qT = persist.tile([128, 4, L], F16, tag="qT")
            kT = persist.tile([128, 4, L], F16, tag="kT")
            v16 = persist.tile([128, 8, HPC, 65], F16, tag="v16")
            attn = persist.tile([128, 4, L], F16, tag="attn")
            nc.vector.memset(v16[:, :, :, 64:65], 1.0)

            def emit_projqk(dg):
                # q/k projections + rope for dim-group dg (128 dims, 2 heads)
                for name in ("q", "k"):
                    dest = qT if name == "q" else kT
                    ct = csp.tile([128, L], F16, tag=f"c{name}")
                    st = csp.tile([128, L], F16, tag=f"s{name}")
                    nc.sync.dma_start(
                        out=ct, in_=cs_src["cq" if name == "q" else "ck"][:, dg, :])
                    nc.sync.dma_start(
                        out=st, in_=cs_src["sq" if name == "q" else "sk"][:, dg, :])
                    for lc in range(2):
                        ls = slice(lc * 512, (lc + 1) * 512)
                        ps = pq.tile([128, 512], F32, tag="pq")
                        for kc in range(8):
                            nc.tensor.matmul(
                                ps, w_sb[name][:, kc, dg * 128:(dg + 1) * 128],
                                x_sb[name][:, kc, ls],
                                start=(kc == 0), stop=(kc == 7))
                        t1 = rscr.tile([128, 512], F16, tag="t1")
                        nc.vector.tensor_mul(t1, ps, ct[:, ls])
                        # z = ps * sin (sin sign-folded AND pre-swapped on
                        # host); t2 = partner-swap of z via Pool DMA copies
                        z = rscr.tile([128, 512], F16, tag="z")
                        nc.vector.tensor_mul(z, ps, st[:, ls])
                        t2 = rscr.tile([128, 512], F16, tag="t2")
                        for a in (0, 64):
                            nc.gpsimd.dma_start(
                                out=t2[a:a + 32, :], in_=z[a + 32:a + 64, :])
                            nc.gpsimd.dma_start(
                                out=t2[a + 32:a + 64, :], in_=z[a:a + 32, :])
                        nc.vector.tensor_add(dest[:, dg, ls], t1, t2)

            def emit_projv():
                # v projection: all 512 dims at once, [s-rows, dims] layout
                for st_i in range(8):
                    ps = pq.tile([128, 512], F32, tag="pq")
                    for kc in range(8):
                        nc.tensor.matmul(
                            ps, x_sb["v"][:, kc, st_i * 128:(st_i + 1) * 128],
                            w_sb["v"][:, kc, :],
                            start=(kc == 0), stop=(kc == 7))
                    for h in range(HPC):
                        if h % 2 == 0:
                            nc.scalar.copy(
                                v16[:, st_i, h, 0:64],
                                ps[:, h * 64:(h + 1) * 64])
                        else:
                            nc.vector.tensor_copy(
                                v16[:, st_i, h, 0:64],
                                ps[:, h * 64:(h + 1) * 64])

            def emit_qk(h):
                # main + mem logits and exps for head h
                dg, ho = h // 2, (h % 2) * 64
                wxs = []
                colsum2 = small.tile([128, 8, 2], F32, tag="colsum2")
                for sc in range(8):
                    for lc in range(2):
                        pw = pq.tile([128, 512], F32, tag="pq")
                        nc.tensor.matmul(
                            pw,
                            kT[ho:ho + 64, dg, sc * 128:(sc + 1) * 128],
                            qT[ho:ho + 64, dg, lc * 512:(lc + 1) * 512],
                            start=True, stop=True)
                        wx = wexpp.tile([128, 512], F16, tag="wx")
                        nc.scalar.activation(
                            wx, pw, AF.Exp,
                            accum_out=colsum2[:, sc, lc:lc + 1])
                        wxs.append(wx)
                wxm = []
                for mc in range(4):
                    for lc in range(2):
                        pw = pq.tile([128, 512], F32, tag="pq")
                        nc.tensor.matmul(
                            pw,
                            kmem_sb[ho:ho + 64, dg, mc * 128:(mc + 1) * 128],
                            qT[ho:ho + 64, dg, lc * 512:(lc + 1) * 512],
                            start=True, stop=True)
                        wx = wexpp.tile([128, 512], F16, tag="wx")
                        nc.scalar.activation(wx, pw, AF.Exp)
                        wxm.append(wx)
                return wxs, wxm, colsum2

            def emit_av(h, wxs, wxm, colsum2):
                dg, ho = h // 2, (h % 2) * 64
                colsum = small.tile([128, 8], F32, tag="colsum")
                nc.vector.tensor_add(
                    colsum, colsum2[:, :, 0], colsum2[:, :, 1])
                rcall = small.tile([128, 8], F32, tag="rcall")
                nc.vector.reciprocal_approx_fast(out=rcall, in_=colsum)
                pmain = pmainp.tile([65, L], F32, tag="pmain")
                for sc in range(8):
                    vs = small.tile([128, 65], F16, tag="vs")
                    nc.vector.tensor_scalar_mul(
                        vs, v16[:, sc, h, :], rcall[:, sc:sc + 1])
                    for lc in range(2):
                        nc.tensor.matmul(
                            pmain[:, lc * 512:(lc + 1) * 512],
                            vs, wxs[sc * 2 + lc],
                            start=(sc == 0), stop=(sc == 7))
                pmem = pmemp.tile([65, L], F32, tag="pmem")
                for mc in range(4):
                    for lc in range(2):
                        nc.tensor.matmul(
                            pmem[:, lc * 512:(lc + 1) * 512],
                            vm_sb[:, h, mc, :], wxm[mc * 2 + lc],
                            start=(mc == 0), stop=(mc == 3))
                # renorm + combine: attn_h = pmain[:64]/den1 + pmem[:64]/den2
                # (den rows must be copied out of PSUM first: the custom-DVE
                # reciprocal's bit ops need raw fp32, PSUM reads convert)
                cd1 = rows.tile([1, L], F32, tag="cd1")
                nc.scalar.copy(cd1, pmain[64:65, :])
                cd2 = rows.tile([1, L], F32, tag="cd2")
                nc.vector.tensor_copy(cd2, pmem[64:65, :])
                r1 = rows.tile([1, L], F32, tag="r1")
                nc.vector.reciprocal_approx_fast(out=r1, in_=cd1)
                r2 = rows.tile([1, L], F32, tag="r2")
                nc.vector.reciprocal_approx_fast(out=r2, in_=cd2)
                bc1 = bcp.tile([64, L], F32, tag="bc1")
                nc.gpsimd.partition_broadcast(bc1, r1)
                bc2 = bcp.tile([64, L], F32, tag="bc2")
                nc.gpsimd.partition_broadcast(bc2, r2)
                u1 = uscr.tile([64, L], F16, tag="u1")
                nc.vector.tensor_mul(u1, pmain[0:64, :], bc1)
                u2 = uscr.tile([64, L], F16, tag="u2")
                nc.vector.tensor_mul(u2, pmem[0:64, :], bc2)
                nc.vector.tensor_add(attn[ho:ho + 64, dg, :], u1, u2)
                if dbg:
                    nc.sync.dma_start(out=dbg_t["dbg_colsum"][:, h, :], in_=colsum)
                    nc.sync.dma_start(out=dbg_t["dbg_rcall"][:, h, :], in_=rcall)
                    nc.sync.dma_start(out=dbg_t["dbg_r1"][:, h, :], in_=r1)
                    nc.sync.dma_start(out=dbg_t["dbg_r2"][:, h, :], in_=r2)
                    if h == 0:
                        pms = uscr.tile([65, L], F32, tag="dbgpm")
                        nc.vector.tensor_copy(pms, pmain)
                        nc.sync.dma_start(out=dbg_t["dbg_pmain"], in_=pms)
                        pms2 = uscr.tile([65, L], F32, tag="dbgpm2")
                        nc.vector.tensor_copy(pms2, pmem)
                        nc.sync.dma_start(out=dbg_t["dbg_pmem"], in_=pms2)
                        nc.sync.dma_start(out=dbg_t["dbg_wx"], in_=wxs[0])

            def emit_outproj():
                for oc in range(8):
                    for lc in range(2):
                        po = pq.tile([128, 512], F32, tag="pq")
                        for dg in range(4):
                            nc.tensor.matmul(
                                po, wo_sb[:, dg, oc * 128:(oc + 1) * 128],
                                attn[:, dg, lc * 512:(lc + 1) * 512],
                                start=(dg == 0), stop=(dg == 3))
                        so = ostage.tile([128, 512], F16, tag="so")
                        if (oc + lc) % 2 == 0:
                            nc.vector.tensor_copy(so, po)
                        else:
                            nc.scalar.copy(so, po)
                        nc.sync.dma_start(
                            out=outT[:, oc, lc * 512:(lc + 1) * 512], in_=so)

            # ---- emission: software-pipelined so PE never waits on exp ----
            emit_projqk(0)
            emit_projv()
            emit_projqk(1)
            pend = emit_qk(0)
            emit_projqk(2)
            for h in range(HPC):
                if h + 1 < HPC:
                    nxt = emit_qk(h + 1)
                else:
                    nxt = None
                if h == 2:
                    emit_projqk(3)
                emit_av(h, *pend)
                pend = nxt
            if dbg:
                nc.sync.dma_start(out=dbg_t["dbg_q"], in_=qT)
                nc.sync.dma_start(out=dbg_t["dbg_k"], in_=kT)
                nc.sync.dma_start(out=dbg_t["dbg_v"], in_=v16)
                nc.sync.dma_start(out=dbg_t["dbg_attn"], in_=attn)
            emit_outproj()
    nc.compile()
    return nc


def _perm64():
    p = np.empty(64, np.int64)
    p[:32] = np.arange(0, 64, 2)
    p[32:] = np.arange(1, 64, 2)
    return p


def _chunk(a, nchunk):
    # [C*128, F] -> [128, C, F]
    c128, f = a.shape
    return np.ascontiguousarray(
        a.reshape(nchunk, 128, f).transpose(1, 0, 2)).astype(NPF16)


def _prep_inputs(inputs):
    """Host-side shard prep. Returns list of per-core input dicts."""
    f = np.float32
    query = np.asarray(inputs["query"], f)
    key = np.asarray(inputs["key"], f)
    value = np.asarray(inputs["value"], f)
    W = np.asarray(inputs["in_proj_weight"], f)
    wo = np.asarray(inputs["out_proj_weight"], f)
    qp = np.asarray(inputs["qp"], f)
    kvp = np.asarray(inputs["kvp"], f)
    k_mem = np.asarray(inputs["k_mem"], f)
    v_mem = np.asarray(inputs["v_mem"], f)
    gate = np.asarray(inputs["gate_attn"], f)
    mask = np.asarray(inputs["mem_mask"]).astype(f)

    g = 1.0 / (1.0 + np.exp(-gate))
    p64 = _perm64()
    sgn = np.tile(np.concatenate(
        [np.full(32, -1.0, f), np.full(32, 1.0, f)]), HPC)

    # per-batch x, shared by the two cores of each batch
    xs = {}
    for n in range(N):
        xs[n] = tuple(
            _chunk(np.ascontiguousarray(t[:, n, :].T), 8)
            for t in (query, key, value))

    def swap32(x):
        y = np.empty_like(x)
        for hb in range(HPC):
            b = hb * 64
            y[b:b + 32] = x[b + 32:b + 64]
            y[b + 32:b + 64] = x[b:b + 32]
        return y

    in_maps = []
    for c in range(NC):
        n, grp = c // 2, c % 2
        dims = np.arange(grp * DCC, (grp + 1) * DCC)
        dims_perm = np.concatenate([dims[h * 64 + p64] for h in range(HPC)])
        gv = np.concatenate(
            [np.full(64, 1.0 - g[grp * HPC + h], f) for h in range(HPC)])

        wq_c = _chunk(np.ascontiguousarray(
            (W[:E][dims_perm] * np.float32(D ** -0.5)).T), 8)
        wk_c = _chunk(np.ascontiguousarray(W[E:2 * E][dims_perm].T), 8)
        wv_c = _chunk(np.ascontiguousarray(
            (W[2 * E:][dims] * gv[:, None]).T), 8)
        wo_c = _chunk(np.ascontiguousarray(wo[:, dims].T), 4)

        cq = _chunk(np.ascontiguousarray(qp[n][:, dims_perm, 0].T), 4)
        sq = _chunk(swap32(qp[n][:, dims_perm, 1].T * sgn[:, None]), 4)
        ck = _chunk(np.ascontiguousarray(kvp[n][:, dims_perm, 0].T), 4)
        sk = _chunk(swap32(kvp[n][:, dims_perm, 1].T * sgn[:, None]), 4)

        km = _chunk(np.ascontiguousarray(k_mem[n][dims_perm, :]), 4)

        vma = np.empty((HPC, 4, 128, 65), f)
        for h in range(HPC):
            vmh = (v_mem[n][dims[h * 64:(h + 1) * 64], :].T
                   * g[grp * HPC + h] * mask[n][:, None])      # [M, 64]
            vma[h, :, :, :64] = vmh.reshape(4, 128, 64)
            vma[h, :, :, 64] = mask[n].reshape(4, 128)
        vm_dev = np.ascontiguousarray(
            vma.transpose(2, 0, 1, 3)).astype(NPF16)           # [128,H,4,65]

        xq_c, xk_c, xv_c = xs[n]
        in_maps.append({
            "xq": xq_c, "xk": xk_c, "xv": xv_c,
            "wq": wq_c, "wk": wk_c, "wv": wv_c, "wo": wo_c,
            "cosq": cq, "sinq": sq, "cosk": ck, "sink": sk,
            "kmem": km, "vm": vm_dev,
        })
    return in_maps


def kernel(**inputs):
    if "nc" not in _COMPILED:
        _COMPILED["nc"] = _build()
    nc = _COMPILED["nc"]
    in_maps = _prep_inputs(inputs)
    res = bass_utils.run_bass_kernel_spmd(nc, in_maps, core_ids=list(range(NC)))
    out = np.zeros((L, N, E), np.float64)
    for c, r in enumerate(res.results):
        n = c // 2
        oc = r["outT"].astype(np.float64)          # [128, 8, L]
        out[:, n, :] += oc.transpose(2, 1, 0).reshape(L, E)
    out = out.astype(np.float32) + np.asarray(inputs["out_proj_bias"],
                                              np.float32)
    return out


# revision 30
# speedup vs baseline: 1.1781x; 1.1131x over previous
"""Trainium2 Bass kernel for nn_Encoder_79585743995180 (sparse_attention).

v2 — batch x head-group sharding. Core c -> (batch n = c//2, head-group
g = c%2 owning 8 heads / 512 dims). vs v1 (head-only sharding):
  - per-core x DMA drops 24MB -> 6MB (each core reads only its batch);
  - projections/attention/outproj all for 8 heads of one batch;
  - renorm path rebuilt: reciprocal_approx_fast (5x faster than
    InstReciprocal), Pool partition_broadcast instead of DRAM-roundtrip
    broadcast DMAs, no [1,1024] multiplies;
  - matmul stream ordered so the PE never waits on the softmax chain
    (QK/memQK of head h+1 emitted before AV of head h).

All matmul operands fp16 (fp8 validated too lossy: >2e-2). fp32 PSUM.
Math per head (validated in numpy, rel err ~7.7e-4):
  z[s,l] = k_h^T q_h   (q pre-scaled by D^-0.5, rope'd)
  wx = exp(z)
  colsum[s] = sum_l wx ; rcall = 1/colsum
  vs[s,:] = [v_h[s,:] , 1] * rcall   -> AV gives numer[d,l], den[l]
  mem path: wxm = exp(zm), vm cols carry gate*mask, col 64 = mask
  attn_h = numer/den + numer_m/den_m   (division via bc'd reciprocal rows)
out_core[e,l] = sum_dc wo[dc,e] attn[dc,l]; host sums the 2 cores per batch.
"""

import numpy as np

import concourse.bacc as bacc
import concourse.mybir as mybir
import concourse.tile as tile
from concourse import bass_utils

F32 = mybir.dt.float32
F16 = mybir.dt.float16
NPF16 = np.float16
AF = mybir.ActivationFunctionType

L = 1024
S = 1024
N = 4
E = 1024
H = 16
D = 64
M = 512
NC = 8
HPC = 8              # heads per core
DCC = HPC * D        # 512 dims per core

_COMPILED = {}


def _build(dbg=False):
    nc = bacc.Bacc("TRN2", target_bir_lowering=False, debug=False)

    # ---- DRAM I/O (all host-prechunked to [128, ...] partition layouts) ----
    xq = nc.dram_tensor("xq", [128, 8, L], F16, kind="ExternalInput").ap()
    xk = nc.dram_tensor("xk", [128, 8, L], F16, kind="ExternalInput").ap()
    xv = nc.dram_tensor("xv", [128, 8, L], F16, kind="ExternalInput").ap()
    wq = nc.dram_tensor("wq", [128, 8, DCC], F16, kind="ExternalInput").ap()
    wk = nc.dram_tensor("wk", [128, 8, DCC], F16, kind="ExternalInput").ap()
    wv = nc.dram_tensor("wv", [128, 8, DCC], F16, kind="ExternalInput").ap()
    wo = nc.dram_tensor("wo", [128, 4, E], F16, kind="ExternalInput").ap()
    cosq = nc.dram_tensor("cosq", [128, 4, L], F16, kind="ExternalInput").ap()
    sinq = nc.dram_tensor("sinq", [128, 4, L], F16, kind="ExternalInput").ap()
    cosk = nc.dram_tensor("cosk", [128, 4, L], F16, kind="ExternalInput").ap()
    sink = nc.dram_tensor("sink", [128, 4, L], F16, kind="ExternalInput").ap()
    kmem = nc.dram_tensor("kmem", [128, 4, M], F16, kind="ExternalInput").ap()
    vm = nc.dram_tensor("vm", [128, HPC, 4, 65], F16, kind="ExternalInput").ap()
    outT = nc.dram_tensor("outT", [128, 8, L], F16, kind="ExternalOutput").ap()
    dbg_t = {}
    if dbg:
        for nm, shp, dt in (("dbg_q", [128, 4, L], F16),
                            ("dbg_k", [128, 4, L], F16),
                            ("dbg_v", [128, 8, HPC, 65], F16),
                            ("dbg_attn", [128, 4, L], F16),
                            ("dbg_colsum", [128, HPC, 8], F32),
                            ("dbg_rcall", [128, HPC, 8], F32),
                            ("dbg_r1", [1, HPC, L], F32),
                            ("dbg_r2", [1, HPC, L], F32),
                            ("dbg_pmain", [65, L], F32),
                            ("dbg_pmem", [65, L], F32),
                            ("dbg_wx", [128, L], F16)):
            dbg_t[nm] = nc.dram_tensor(nm, shp, dt, kind="ExternalOutput").ap()

    with tile.TileContext(nc) as tc:
        with (
            tc.tile_pool(name="const", bufs=1) as const,
            tc.tile_pool(name="persist", bufs=1) as persist,
            tc.tile_pool(name="wexp", bufs=9 if dbg else 10) as wexpp,
            tc.tile_pool(name="cs", bufs=1 if dbg else 2) as csp,
            tc.tile_pool(name="small", bufs=3) as small,
            tc.tile_pool(name="rows", bufs=1) as rows,
            tc.tile_pool(name="bcp", bufs=1) as bcp,
            tc.tile_pool(name="uscr", bufs=2) as uscr,
            tc.tile_pool(name="rscr", bufs=2 if dbg else 4) as rscr,
            tc.tile_pool(name="ostage", bufs=2 if dbg else 3) as ostage,
            tc.tile_pool(name="pq", bufs=2, space="PSUM") as pq,
            tc.tile_pool(name="pmain", bufs=1, space="PSUM") as pmainp,
            tc.tile_pool(name="pmem", bufs=1, space="PSUM") as pmemp,
        ):
            # ---- constants / activations into SBUF ----
            # DMA issue order matters: the first projection needs wq + xq
            # chunks, so those go first; wo/kmem/vm are needed much later.
            cs_src = {"cq": cosq, "sq": sinq, "ck": cosk, "sk": sink}
            w_sb = {}
            x_sb = {}
            for name, wsrc, xsrc in (("q", wq, xq), ("k", wk, xk),
                                     ("v", wv, xv)):
                wt = const.tile([128, 8, DCC], F16, tag=f"w_{name}")
                nc.sync.dma_start(out=wt, in_=wsrc)
                w_sb[name] = wt
                xt = const.tile([128, 8, L], F16, tag=f"x_{name}")
                for kc in range(8):
                    nc.sync.dma_start(out=xt[:, kc, :], in_=xsrc[:, kc, :])
                x_sb[name] = xt
            kmem_sb = const.tile([128, 4, M], F16, tag="kmem")
            nc.sync.dma_start(out=kmem_sb, in_=kmem)
            vm_sb = const.tile([128, HPC, 4, 65], F16, tag="vm")
            nc.sync.dma_start(out=vm_sb, in_=vm)
            wo_sb = const.tile([128, 4, E], F16, tag="wo")
            nc.sync.dma_start(out=wo_sb, in_=wo)

            qT = persist.tile([128, 4, L], F16, tag="qT")
            kT = persist.tile([128, 4, L], F16, tag="kT")
            v16 = persist.tile([128, 8, HPC, 65], F16, tag="v16")
            attn = persist.tile([128, 4, L], F16, tag="attn")
            nc.vector.memset(v16[:, :, :, 64:65], 1.0)

            def emit_projqk(dg):
                # q/k projections + rope for dim-group dg (128 dims, 2 heads)
                for name in ("q", "k"):
                    dest = qT if name == "q" else kT
                    ct = csp.tile([128, L], F16, tag=f"c{name}")
                    st = csp.tile([128, L], F16, tag=f"s{name}")
                    nc.sync.dma_start(
                        out=ct, in_=cs_src["cq" if name == "q" else "ck"][:, dg, :])
                    nc.sync.dma_start(
                        out=st, in_=cs_src["sq" if name == "q" else "sk"][:, dg, :])
                    ps = pq.tile([128, L], F32, tag="pq")
                    for lc in range(2):
                        ls = slice(lc * 512, (lc + 1) * 512)
                        for kc in range(8):
                            nc.tensor.matmul(
                                ps[:, ls],
                                w_sb[name][:, kc, dg * 128:(dg + 1) * 128],
                                x_sb[name][:, kc, ls],
                                start=(kc == 0), stop=(kc == 7))
                    for lc in range(2):
                        ls = slice(lc * 512, (lc + 1) * 512)
                        t1 = rscr.tile([128, 512], F16, tag="t1")
                        nc.vector.tensor_mul(t1, ps[:, ls], ct[:, ls])
                        # z = ps * sin (sin sign-folded AND pre-swapped on
                        # host); t2 = partner-swap of z via Pool DMA copies
                        z = rscr.tile([128, 512], F16, tag="z")
                        nc.vector.tensor_mul(z, ps[:, ls], st[:, ls])
                        t2 = rscr.tile([128, 512], F16, tag="t2")
                        for a in (0, 64):
                            nc.gpsimd.dma_start(
                                out=t2[a:a + 32, :], in_=z[a + 32:a + 64, :])
                            nc.gpsimd.dma_start(
                                out=t2[a + 32:a + 64, :], in_=z[a:a + 32, :])
                        nc.vector.tensor_add(dest[:, dg, ls], t1, t2)

            def emit_projv():
                # v projection: all 512 dims at once, [s-rows, dims] layout;
                # two row-blocks share one [128, 1024] psum tile
                for sp in range(4):
                    ps = pq.tile([128, L], F32, tag="pq")
                    for half in range(2):
                        st_i = sp * 2 + half
                        hs = slice(half * 512, (half + 1) * 512)
                        for kc in range(8):
                            nc.tensor.matmul(
                                ps[:, hs],
                                x_sb["v"][:, kc, st_i * 128:(st_i + 1) * 128],
                                w_sb["v"][:, kc, :],
                                start=(kc == 0), stop=(kc == 7))
                        for h in range(HPC):
                            if h % 2 == 0:
                                nc.scalar.copy(
                                    v16[:, st_i, h, 0:64],
                                    ps[:, half * 512 + h * 64:
                                       half * 512 + (h + 1) * 64])
                            else:
                                nc.vector.tensor_copy(
                                    v16[:, st_i, h, 0:64],
                                    ps[:, half * 512 + h * 64:
                                       half * 512 + (h + 1) * 64])

            def emit_qk(h):
                # main + mem logits and exps for head h; one [128, 1024]
                # psum tile (= 2 matmuls) per exp halves ACT overhead and
                # accumulator reads
                dg, ho = h // 2, (h % 2) * 64
                wxs = []
                colsum = small.tile([128, 8], F32, tag="colsum")
                for sc in range(8):
                    pw = pq.tile([128, L], F32, tag="pq")
                    for lc in range(2):
                        nc.tensor.matmul(
                            pw[:, lc * 512:(lc + 1) * 512],
                            kT[ho:ho + 64, dg, sc * 128:(sc + 1) * 128],
                            qT[ho:ho + 64, dg, lc * 512:(lc + 1) * 512],
                            start=True, stop=True)
                    wx = wexpp.tile([128, L], F16, tag="wx")
                    nc.scalar.activation(
                        wx, pw, AF.Exp, accum_out=colsum[:, sc:sc + 1])
                    wxs.append(wx)
                wxm = []
                for mc in range(4):
                    pw = pq.tile([128, L], F32, tag="pq")
                    for lc in range(2):
                        nc.tensor.matmul(
                            pw[:, lc * 512:(lc + 1) * 512],
                            kmem_sb[ho:ho + 64, dg, mc * 128:(mc + 1) * 128],
                            qT[ho:ho + 64, dg, lc * 512:(lc + 1) * 512],
                            start=True, stop=True)
                    wx = wexpp.tile([128, L], F16, tag="wx")
                    nc.scalar.activation(wx, pw, AF.Exp)
                    wxm.append(wx)
                return wxs, wxm, colsum

            def emit_av(h, wxs, wxm, colsum):
                dg, ho = h // 2, (h % 2) * 64
                rcall = small.tile([128, 8], F32, tag="rcall")
                nc.vector.reciprocal_approx_fast(out=rcall, in_=colsum)
                pmain = pmainp.tile([65, L], F32, tag="pmain")
                for sc in range(8):
                    vs = small.tile([128, 65], F16, tag="vs")
                    nc.vector.tensor_scalar_mul(
                        vs, v16[:, sc, h, :], rcall[:, sc:sc + 1])
                    for lc in range(2):
                        nc.tensor.matmul(
                            pmain[:, lc * 512:(lc + 1) * 512],
                            vs, wxs[sc][:, lc * 512:(lc + 1) * 512],
                            start=(sc == 0), stop=(sc == 7))
                pmem = pmemp.tile([65, L], F32, tag="pmem")
                for mc in range(4):
                    for lc in range(2):
                        nc.tensor.matmul(
                            pmem[:, lc * 512:(lc + 1) * 512],
                            vm_sb[:, h, mc, :],
                            wxm[mc][:, lc * 512:(lc + 1) * 512],
                            start=(mc == 0), stop=(mc == 3))
                # renorm + combine: attn_h = pmain[:64]/den1 + pmem[:64]/den2
                # (den rows must be copied out of PSUM first: the custom-DVE
                # reciprocal's bit ops need raw fp32, PSUM reads convert)
                cd1 = rows.tile([1, L], F32, tag="cd1")
                nc.scalar.copy(cd1, pmain[64:65, :])
                cd2 = rows.tile([1, L], F32, tag="cd2")
                nc.vector.tensor_copy(cd2, pmem[64:65, :])
                r1 = rows.tile([1, L], F32, tag="r1")
                nc.vector.reciprocal_approx_fast(out=r1, in_=cd1)
                r2 = rows.tile([1, L], F32, tag="r2")
                nc.vector.reciprocal_approx_fast(out=r2, in_=cd2)
                bc1 = bcp.tile([64, L], F32, tag="bc1")
                nc.gpsimd.partition_broadcast(bc1, r1)
                bc2 = bcp.tile([64, L], F32, tag="bc2")
                nc.gpsimd.partition_broadcast(bc2, r2)
                u1 = uscr.tile([64, L], F16, tag="u1")
                nc.vector.tensor_mul(u1, pmain[0:64, :], bc1)
                u2 = uscr.tile([64, L], F16, tag="u2")
                nc.vector.tensor_mul(u2, pmem[0:64, :], bc2)
                nc.vector.tensor_add(attn[ho:ho + 64, dg, :], u1, u2)
                if dbg:
                    nc.sync.dma_start(out=dbg_t["dbg_colsum"][:, h, :], in_=colsum)
                    nc.sync.dma_start(out=dbg_t["dbg_rcall"][:, h, :], in_=rcall)
                    nc.sync.dma_start(out=dbg_t["dbg_r1"][:, h, :], in_=r1)
                    nc.sync.dma_start(out=dbg_t["dbg_r2"][:, h, :], in_=r2)
                    if h == 0:
                        pms = uscr.tile([65, L], F32, tag="dbgpm")
                        nc.vector.tensor_copy(pms, pmain)
                        nc.sync.dma_start(out=dbg_t["dbg_pmain"], in_=pms)
                        pms2 = uscr.tile([65, L], F32, tag="dbgpm2")
                        nc.vector.tensor_copy(pms2, pmem)
                        nc.sync.dma_start(out=dbg_t["dbg_pmem"], in_=pms2)
                        nc.sync.dma_start(out=dbg_t["dbg_wx"], in_=wxs[0])

            def emit_outproj():
                for oc in range(8):
                    po = pq.tile([128, L], F32, tag="pq")
                    for lc in range(2):
                        ls = slice(lc * 512, (lc + 1) * 512)
                        for dg in range(4):
                            nc.tensor.matmul(
                                po[:, ls],
                                wo_sb[:, dg, oc * 128:(oc + 1) * 128],
                                attn[:, dg, ls],
                                start=(dg == 0), stop=(dg == 3))
                    so = ostage.tile([128, L], F16, tag="so")
                    if oc % 2 == 0:
                        nc.vector.tensor_copy(so, po)
                    else:
                        nc.scalar.copy(so, po)
                    nc.sync.dma_start(out=outT[:, oc, :], in_=so)

            # ---- emission: software-pipelined so PE never waits on exp ----
            emit_projqk(0)
            emit_projv()
            emit_projqk(1)
            pend = emit_qk(0)
            emit_projqk(2)
            for h in range(HPC):
                if h + 1 < HPC:
                    nxt = emit_qk(h + 1)
                else:
                    nxt = None
                if h == 2:
                    emit_projqk(3)
                emit_av(h, *pend)
                pend = nxt
            if dbg:
                nc.sync.dma_start(out=dbg_t["dbg_q"], in_=qT)
                nc.sync.dma_start(out=dbg_t["dbg_k"], in_=kT)
                nc.sync.dma_start(out=dbg_t["dbg_v"], in_=v16)
                nc.sync.dma_start(out=dbg_t["dbg_attn"], in_=attn)
            emit_outproj()
    nc.compile()
    return nc


def _perm64():
    p = np.empty(64, np.int64)
    p[:32] = np.arange(0, 64, 2)
    p[32:] = np.arange(1, 64, 2)
    return p


def _chunk(a, nchunk):
    # [C*128, F] -> [128, C, F]
    c128, f = a.shape
    return np.ascontiguousarray(
        a.reshape(nchunk, 128, f).transpose(1, 0, 2)).astype(NPF16)


def _prep_inputs(inputs):
    """Host-side shard prep. Returns list of per-core input dicts."""
    f = np.float32
    query = np.asarray(inputs["query"], f)
    key = np.asarray(inputs["key"], f)
    value = np.asarray(inputs["value"], f)
    W = np.asarray(inputs["in_proj_weight"], f)
    wo = np.asarray(inputs["out_proj_weight"], f)
    qp = np.asarray(inputs["qp"], f)
    kvp = np.asarray(inputs["kvp"], f)
    k_mem = np.asarray(inputs["k_mem"], f)
    v_mem = np.asarray(inputs["v_mem"], f)
    gate = np.asarray(inputs["gate_attn"], f)
    mask = np.asarray(inputs["mem_mask"]).astype(f)

    g = 1.0 / (1.0 + np.exp(-gate))
    p64 = _perm64()
    sgn = np.tile(np.concatenate(
        [np.full(32, -1.0, f), np.full(32, 1.0, f)]), HPC)

    # per-batch x, shared by the two cores of each batch
    xs = {}
    for n in range(N):
        xs[n] = tuple(
            _chunk(np.ascontiguousarray(t[:, n, :].T), 8)
            for t in (query, key, value))

    def swap32(x):
        y = np.empty_like(x)
        for hb in range(HPC):
            b = hb * 64
            y[b:b + 32] = x[b + 32:b + 64]
            y[b + 32:b + 64] = x[b:b + 32]
        return y

    in_maps = []
    for c in range(NC):
        n, grp = c // 2, c % 2
        dims = np.arange(grp * DCC, (grp + 1) * DCC)
        dims_perm = np.concatenate([dims[h * 64 + p64] for h in range(HPC)])
        gv = np.concatenate(
            [np.full(64, 1.0 - g[grp * HPC + h], f) for h in range(HPC)])

        wq_c = _chunk(np.ascontiguousarray(
            (W[:E][dims_perm] * np.float32(D ** -0.5)).T), 8)
        wk_c = _chunk(np.ascontiguousarray(W[E:2 * E][dims_perm].T), 8)
        wv_c = _chunk(np.ascontiguousarray(
            (W[2 * E:][dims] * gv[:, None]).T), 8)
        wo_c = _chunk(np.ascontiguousarray(wo[:, dims].T), 4)

        cq = _chunk(np.ascontiguousarray(qp[n][:, dims_perm, 0].T), 4)
        sq = _chunk(swap32(qp[n][:, dims_perm, 1].T * sgn[:, None]), 4)
        ck = _chunk(np.ascontiguousarray(kvp[n][:, dims_perm, 0].T), 4)
        sk = _chunk(swap32(kvp[n][:, dims_perm, 1].T * sgn[:, None]), 4)

        km = _chunk(np.ascontiguousarray(k_mem[n][dims_perm, :]), 4)

        vma = np.empty((HPC, 4, 128, 65), f)
        for h in range(HPC):
            vmh = (v_mem[n][dims[h * 64:(h + 1) * 64], :].T
                   * g[grp * HPC + h] * mask[n][:, None])      # [M, 64]
            vma[h, :, :, :64] = vmh.reshape(4, 128, 64)
            vma[h, :, :, 64] = mask[n].reshape(4, 128)
        vm_dev = np.ascontiguousarray(
            vma.transpose(2, 0, 1, 3)).astype(NPF16)           # [128,H,4,65]

        xq_c, xk_c, xv_c = xs[n]
        in_maps.append({
            "xq": xq_c, "xk": xk_c, "xv": xv_c,
            "wq": wq_c, "wk": wk_c, "wv": wv_c, "wo": wo_c,
            "cosq": cq, "sinq": sq, "cosk": ck, "sink": sk,
            "kmem": km, "vm": vm_dev,
        })
    return in_maps


def kernel(**inputs):
    if "nc" not in _COMPILED:
        _COMPILED["nc"] = _build()
    nc = _COMPILED["nc"]
    in_maps = _prep_inputs(inputs)
    res = bass_utils.run_bass_kernel_spmd(nc, in_maps, core_ids=list(range(NC)))
    out = np.zeros((L, N, E), np.float64)
    for c, r in enumerate(res.results):
        n = c // 2
        oc = r["outT"].astype(np.float64)          # [128, 8, L]
        out[:, n, :] += oc.transpose(2, 1, 0).reshape(L, E)
    out = out.astype(np.float32) + np.asarray(inputs["out_proj_bias"],
                                              np.float32)
    return out
